# revision 17
# baseline (speedup 1.0000x reference)
"""HDC generic encoder kernel for 8 Trainium2 NeuronCores.

out[b,d] = sum_{w=0..56} K[w,d] * prod_{j=0..6} enc0[b, w+1+j, (d-(6-j)) mod D]

Algorithm (per core, 8 batches, partitions q = b_local*16 + blk):
  With A[p,e] := enc0[p, (e+p) mod D], the window product is
  PROD_w(d) = prod_{p=w+1..w+7} A[p, d-w-7].  Since all values are +/-1
  (x*x = 1), a single cumulative scan S[m] = prod_{p<=m} A[p] gives
  PROD_w = S[w+7] * S[w] -- no division needed.  The keys multiply is
  folded into the table on the host via kappa[m] with
  kappa[w+7]*kappa[w] = K[w] (kappa[m] = K[m-7]*kappa[m-7], kappa[<7]=1),
  so the device computes sum_w Shat[w+7]*Shat[w] directly.

Device pipeline (sim: 46.4us, vs 4172us for the gather+redistribute
baseline; exact output):
  1. 63 per-segment DMAs (alternating SP/Act queues) load the host-packed
     table gin[b, s, blk, :] into G[q, slot s] (stride 704), trimmed to
     the read range u in [s+6, 694).
  2. memset slot 0 = 1.0 (S[0], empty product) on Pool.
  3. In-place scan on DVE: slot m <- slot (m-1) * slot m shifted by m-1.
     Slot m then holds S[m][c], c = tau+70, tau = d'-w-7; columns c < 7
     are never read, so the scan starts at c = 7.
  4. Windowed products BD[w,d'] = S[w+7][d'-w+63] * S[w][d'-w+63] plus
     log-tree sums, chunked over w: Pool takes w 0..34 (its first small
     chunks start ~6us in, overlapping the scan), DVE takes w 35..56
     after the scan.  Per chunk: one strided BD multiply, a log tree of
     adds, one accumulate add (bf16 throughout; sums <= 57 are exact).
  5. Final f32 combine of the two accumulators in two halves, each
     followed immediately by its output DMA (SP / Act).
"""

import numpy as np

import concourse.bacc as bacc
import concourse.bass as bass
import concourse.mybir as mybir
from concourse.bass_utils import run_bass_kernel_spmd
from concourse.tile import TileContext

B, T, F, D = 64, 4, 64, 10000
NGRAMS = 7
W = F - NGRAMS  # 57 windows
NCORES = 8
BPC = B // NCORES  # 8 batches per core
MROWS, HROWS = 3000, 200

NBLK = 16
BLKW = D // NBLK  # 625
SEG2 = 704  # slot stride in G tile
SEGW = 694  # valid row width (u in [0, 694))
GINW = 696  # packed gin row width (694 + pad)
KW2 = 626  # BD row stride

POOL_CHUNKS = [(0, 2), (2, 3), (5, 4), (9, 8), (17, 9), (26, 9)]  # w-chunks on Pool
DVE_CHUNKS = [(35, 8), (43, 8), (51, 6)]  # w-chunks on DVE

_CACHE = {}


def _rv(tile, stride, cnt, off, width):
    """Strided-row view: cnt rows of `width` elems at `off + i*stride`."""
    base = (off // stride) * stride
    o2 = off - base
    assert o2 + width <= stride
    v = tile[:, base : base + cnt * stride].rearrange("p (s k) -> p s k", k=stride)
    return v[:, :, o2 : o2 + width]


def _build_nc():
    nc = bacc.Bacc(None)
    gin = nc.dram_tensor(
        "gin", [BPC, F - 1, NBLK, GINW], mybir.dt.bfloat16, kind="ExternalInput"
    )
    out = nc.dram_tensor("out", [BPC, D], mybir.dt.float32, kind="ExternalOutput")
    out_r = out.rearrange("b (q d) -> (b q) d", d=BLKW)  # [128, 625]

    with TileContext(nc) as tc:
        with tc.tile_pool(name="main", bufs=1) as pool:
            g = pool.tile([128, F * SEG2], mybir.dt.bfloat16, tag="G")
            g3 = g[:, :].rearrange("p (s k) -> p s k", k=SEG2)

            # S[0] = empty product = 1 (on Pool, off the DVE critical path);
            # only c in [63, 688) is ever read from slot 0.
            nc.gpsimd.memset(g3[:, 0, 63:688], 1.0)

            # segment loads, alternating the two DMA-capable idle queues.
            # Slot m is only read at u in [m+6, 694) (slot 1: [7, 693)).
            for s in range(1, F):
                eng = nc.sync if (s % 2) else nc.scalar
                lo, hi = (7, 693) if s == 1 else (s + 6, 694)
                eng.dma_start(
                    out=g3[:, s, lo:hi], in_=gin[:, s - 1, :, lo:hi]
                )

            # in-place cumulative product scan (slot 1 is S[1] as loaded).
            # Columns c < 7 are never read downstream, so start at 7.
            for m in range(2, F):
                wdt = 695 - m - 7
                nc.vector.tensor_mul(
                    g3[:, m, 7 : 7 + wdt],
                    g3[:, m - 1, 7 : 7 + wdt],
                    g3[:, m, m + 6 : m + 6 + wdt],
                )

            # BD + tree sums, chunked over w on two engines
            accP = pool.tile([128, KW2], mybir.dt.bfloat16, tag="accP")
            accD = pool.tile([128, KW2], mybir.dt.bfloat16, tag="accD")
            nc.gpsimd.memset(accP[:, :], 0.0)
            nc.gpsimd.memset(accD[:, :], 0.0)
            bdP = pool.tile([128, 9 * KW2], mybir.dt.bfloat16, tag="bdP")
            bdD = pool.tile([128, 8 * KW2], mybir.dt.bfloat16, tag="bdD")
            HB = BLKW // 2  # 312; split final add + out DMA into halves

            def bd_chunk(eng, bd, acc, w0, wc):
                # BD[w] = S[w+7][c] * S[w][c], c = d'-w+63
                # flat: in0 = w*(SEG2-1) + 7*SEG2+63, in1 = w*(SEG2-1) + 63
                eng.tensor_mul(
                    _rv(bd, KW2, wc, 0, BLKW),
                    _rv(g, SEG2 - 1, wc, w0 * (SEG2 - 1) + 7 * SEG2 + 63, BLKW),
                    _rv(g, SEG2 - 1, wc, w0 * (SEG2 - 1) + 63, BLKW),
                )
                n = wc
                while n > 1:
                    m = n // 2
                    eng.tensor_add(
                        _rv(bd, KW2, m, 0, BLKW),
                        _rv(bd, KW2, m, 0, BLKW),
                        _rv(bd, KW2, m, (n - m) * KW2, BLKW),
                    )
                    n -= m
                eng.tensor_add(acc[:, 0:BLKW], acc[:, 0:BLKW], bd[:, 0:BLKW])

            for w0, wc in POOL_CHUNKS:
                bd_chunk(nc.gpsimd, bdP, accP, w0, wc)
            for w0, wc in DVE_CHUNKS:
                bd_chunk(nc.vector, bdD, accD, w0, wc)

            accF = pool.tile([128, KW2], mybir.dt.float32, tag="accF")
            nc.vector.tensor_add(accF[:, 0:HB], accP[:, 0:HB], accD[:, 0:HB])
            nc.sync.dma_start(out=out_r[:, 0:HB], in_=accF[:, 0:HB])
            nc.vector.tensor_add(accF[:, HB:BLKW], accP[:, HB:BLKW], accD[:, HB:BLKW])
            nc.scalar.dma_start(out=out_r[:, HB:BLKW], in_=accF[:, HB:BLKW])
    nc.compile()
    return nc


def _host_prep(x, keys_weight, motion_table, hr_table):
    import ml_dtypes

    bf16 = ml_dtypes.bfloat16

    x0 = np.asarray(x[:, 0, :], dtype=np.float32)  # [B, F]
    mi = np.clip(
        np.round((x0[:, : F - 1] + 3.0) / 6.0 * (MROWS - 1)).astype(np.int64),
        0,
        MROWS - 1,
    )
    hi = (
        np.clip(
            np.round((x0[:, F - 1] - 50.0) / 150.0 * (HROWS - 1)).astype(np.int64),
            0,
            HROWS - 1,
        )
        + MROWS
    )
    rows = np.concatenate([mi, hi[:, None]], axis=1)  # [B, F] int64

    # table as uint16 bf16 bit patterns; +/-1 multiplies = sign-bit XOR
    tb = np.concatenate(
        [np.asarray(motion_table), np.asarray(hr_table)], axis=0
    ).astype(bf16)
    tb_u = tb.view(np.uint16)  # [3200, D]
    EXT_LO = 69  # cols -69..D-1+? -> ext width D+69
    tbl_ext = np.concatenate([tb_u[:, D - EXT_LO :], tb_u], axis=1)  # [3200, D+69]

    # kappa in int8: kap[m] = roll(K[m-7], -m) * kap[m-7], kap[<7] = 1
    Ki = np.where(np.asarray(keys_weight)[:W] >= 0, 1, -1).astype(np.int8)  # [57, D]
    kap = np.ones((F, D), np.int8)
    for m in range(NGRAMS, F):
        kap[m] = np.roll(Ki[m - NGRAMS], -m) * kap[m - NGRAMS]
    K2 = np.ones((F, D), np.int8)
    for m in range(1, F):
        K2[m] = kap[m] * kap[m - 1]
    K2r = np.empty_like(K2)
    for m in range(F):
        K2r[m] = np.roll(K2[m], m)
    K2r_ext = np.concatenate([K2r[:, D - EXT_LO :], K2r], axis=1)  # [F, D+69]
    sign = (K2r_ext < 0).astype(np.uint16) << 15  # [F, D+69]

    in_maps = []
    for c in range(NCORES):
        rc = rows[BPC * c : BPC * (c + 1)]  # [8, F]
        E = tbl_ext[rc]  # [8, F, D+69] uint16
        gin = np.zeros((BPC, F - 1, NBLK, GINW), np.uint16)
        for blk in range(NBLK):
            c0 = blk * BLKW
            gin[:, :, blk, 0:SEGW] = (
                E[:, 1:, c0 : c0 + SEGW] ^ sign[None, 1:, c0 : c0 + SEGW]
            )
        in_maps.append({"gin": gin.view(bf16)})
    return in_maps


def run(inputs, trace=False):
    if "nc" not in _CACHE:
        _CACHE["nc"] = _build_nc()
    nc = _CACHE["nc"]
    in_maps = _host_prep(**inputs)
    res = run_bass_kernel_spmd(nc, in_maps, core_ids=list(range(NCORES)), trace=trace)
    outs = [res.results[c]["out"] for c in range(NCORES)]
    full = np.concatenate(outs, axis=0).astype(np.float32)
    return full, res


def kernel(**inputs) -> np.ndarray:
    full, _ = run(inputs, trace=False)
    return full
